# revision 4
# baseline (speedup 1.0000x reference)
"""PointerNet kernel for Trainium2 (8 NeuronCores, data-parallel over batch).

The reference decoder computes ``logits = h @ ptr_W.T + ptr_b`` with
``ptr_W`` of shape ``(1, H)``, i.e. logits are ``[B, 1]``, and then takes
``argmax(logits, axis=1)`` — an argmax over a singleton axis, which is
identically 0 for every step, batch row, and input value (the reference
itself notes "always 0, faithful to source"). The returned pointer matrix
is therefore the constant int32 zero tensor of shape [B, S], independent
of all inputs and weights.

The Bass kernel consequently only has to materialize that output: each of
the 8 cores owns B/8 = 32 batch rows and writes a zeroed [32, S] int32
block (memset in SBUF, one DMA out). This is the exact function the
reference computes, at the true memory roofline for the required output
bytes (B*S*4 = 512 KiB total, 64 KiB per core). The [128, 128] flat view
was the fastest descriptor shape in a measured sweep of p x 16384/p
layouts (2.0 us/iter serialized vs 3.1 us for the natural [32, 512]);
splitting the DMA across two HWDGE queues or copying DRAM->DRAM both
measured slower.
"""

import numpy as np

import concourse.bass as bass
from concourse import mybir
from concourse.bass_utils import run_bass_kernel_spmd

B, S = 256, 512
N_CORES = 8
BP = B // N_CORES  # 32 batch rows per core

_NC_CACHE = None


def _build_program():
    # Output viewed flat as [128 partitions x 128 int32] so the memset uses
    # all DVE lanes and the DMA engages all SBUF partitions. The DVE memset
    # signals a semaphore; the SP (sync) engine's HWDGE DMA waits on it —
    # same-engine memset->dma_start is NOT ordered (DGE issue races ahead of
    # compute retire), so the cross-engine semaphore is required.
    nc = bass.Bass(target_bir_lowering=False)
    out = nc.dram_tensor("out", [BP, S], mybir.dt.int32, kind="ExternalOutput")
    with (
        nc.Block() as block,
        nc.semaphore("set_sem") as set_sem,
        nc.semaphore("dma_sem") as dma_sem,
        nc.sbuf_tensor("z", [128, 128], mybir.dt.int32) as z,
    ):

        @block.vector
        def _(vector):
            vector.memset(z[:, :], 0).then_inc(set_sem, 1)

        @block.sync
        def _(sync):
            sync.wait_ge(set_sem, 1)
            sync.dma_start(
                bass.AP(out, 0, [[128, 128], [1, 128]]), z[:, :]
            ).then_inc(dma_sem, 16)
            sync.wait_ge(dma_sem, 16)

    return nc


def kernel(**inputs: np.ndarray) -> np.ndarray:
    global _NC_CACHE
    if _NC_CACHE is None:
        _NC_CACHE = _build_program()
    nc = _NC_CACHE

    in_maps = [{} for _ in range(N_CORES)]
    res = run_bass_kernel_spmd(nc, in_maps, core_ids=list(range(N_CORES)))
    parts = [np.asarray(r["out"], dtype=np.int32) for r in res.results]
    return np.concatenate(parts, axis=0)


# revision 6
# speedup vs baseline: 28982.6432x; 28982.6432x over previous
"""PointerNet kernel for Trainium2 (8 NeuronCores, data-parallel over batch).

The reference decoder computes ``logits = h @ ptr_W.T + ptr_b`` with
``ptr_W`` of shape ``(1, H)``, i.e. logits are ``[B, 1]``, and then takes
``argmax(logits, axis=1)`` — an argmax over a singleton axis, which is
identically 0 for every step, batch row, and input value (the reference
itself notes "always 0, faithful to source"). The returned pointer matrix
is therefore the constant int32 zero tensor of shape [B, S], independent
of all inputs and weights.

The Bass kernel consequently only has to materialize that output: each of
the 8 cores owns B/8 = 32 batch rows and writes a zeroed [32, S] int32
block (memset in SBUF, one DMA out). This is the exact function the
reference computes, at the true memory roofline for the required output
bytes (B*S*4 = 512 KiB total, 64 KiB per core). The [64, 256] flat view
was the best descriptor shape in measured sweeps of p x 16384/p layouts:
~2.4 us/iter serialized and reproducible to +/-50 ns across runs, vs
3.1 us for the natural [32, 512] and a high-variance 2.0-2.9 us for
[128, 128]; splitting the DMA across two HWDGE queues, gpsimd SW-DGE,
and DRAM->DRAM copies all measured slower (or wedged the exec unit).
"""

import numpy as np

import concourse.bass as bass
from concourse import mybir
from concourse.bass_utils import run_bass_kernel_spmd

B, S = 256, 512
N_CORES = 8
BP = B // N_CORES  # 32 batch rows per core

_NC_CACHE = None


def _build_program():
    # Output viewed flat as [64 partitions x 256 int32] (the measured-best
    # DMA descriptor shape). The DVE memset signals a semaphore; the SP
    # (sync) engine's HWDGE DMA waits on it — same-engine memset->dma_start
    # is NOT ordered (DGE issue races ahead of compute retire), so the
    # cross-engine semaphore is required. Only gpsimd/SP/Activation can
    # initiate DMAs, so a single-engine variant is impossible (gpsimd's
    # SW DGE wedges the exec unit under load).
    nc = bass.Bass(target_bir_lowering=False)
    out = nc.dram_tensor("out", [BP, S], mybir.dt.int32, kind="ExternalOutput")
    with (
        nc.Block() as block,
        nc.semaphore("set_sem") as set_sem,
        nc.semaphore("dma_sem") as dma_sem,
        nc.sbuf_tensor("z", [64, 256], mybir.dt.int32) as z,
    ):

        @block.vector
        def _(vector):
            vector.memset(z[:, :], 0).then_inc(set_sem, 1)

        @block.sync
        def _(sync):
            sync.wait_ge(set_sem, 1)
            sync.dma_start(
                bass.AP(out, 0, [[256, 64], [1, 256]]), z[:, :]
            ).then_inc(dma_sem, 16)
            sync.wait_ge(dma_sem, 16)

    return nc


def kernel(**inputs: np.ndarray) -> np.ndarray:
    global _NC_CACHE
    if _NC_CACHE is None:
        _NC_CACHE = _build_program()
    nc = _NC_CACHE

    in_maps = [{} for _ in range(N_CORES)]
    res = run_bass_kernel_spmd(nc, in_maps, core_ids=list(range(N_CORES)))
    parts = [np.asarray(r["out"], dtype=np.int32) for r in res.results]
    return np.concatenate(parts, axis=0)
